# revision 24
# baseline (speedup 1.0000x reference)
"""Bass/Trainium2 kernel for nn_D_constraint1: 0.001*sqrt(sum_i (||d_i||^2 - 1)^2).

Sharding: d [16384, 2048] is split row-wise across 8 NeuronCores (2048 rows
each; the row dimension is fully parallel per the sharding hint). Each core
streams its 16 MiB shard HBM->SBUF in 16 [128,2048] tiles and computes per-row
sums of squares on the fly: odd tiles on the scalar engine (Square activation
with free-axis accumulator), even tiles on the vector engine (square then
pool-average). Two more scalar-engine activations fold the per-row sums into
sum (1-s)^2 per partition. The host gathers the per-core partials, sums,
takes sqrt and scales - the scalar "all-reduce" of the sharding hint.
"""

from contextlib import ExitStack

import numpy as np

import concourse.bass as bass
from concourse import bacc, mybir
from concourse.bass_utils import run_bass_kernel_spmd

N, K = 16384, 2048
NCORES = 8
R = N // NCORES  # rows per core
P = 128          # SBUF partitions
T = R // P       # row-tiles per core

_nc_cache = None


def _build_nc_v3(wait_out="wait", reduce_mode="none"):
    f32 = mybir.dt.float32
    nc = bacc.Bacc("TRN2", target_bir_lowering=False, debug=False)
    d = nc.dram_tensor("d", [R, K], f32, kind="ExternalInput").ap()
    out_shape = [1, 2] if reduce_mode == "gpsimd" else [P, 2]
    out = nc.dram_tensor("out", out_shape, f32, kind="ExternalOutput").ap()
    Square = mybir.ActivationFunctionType.Square

    act_tiles = list(range(1, T, 2))  # ACT gets the last tile (shorter tail)
    dve_tiles = list(range(0, T, 2))
    NA, NV = len(act_tiles), len(dve_tiles)

    ctx = ExitStack()
    dsem = [ctx.enter_context(nc.semaphore(f"dma_{i}")) for i in range(T)]
    with (
        ctx,
        nc.semaphore("act_sem") as act_sem,
        nc.semaphore("dve_sem") as dve_sem,
        nc.semaphore("pool_sem") as pool_sem,
        nc.semaphore("outd_sem") as outd_sem,
        nc.semaphore("dummy_sem") as dummy_sem,
        nc.sbuf_tensor("t", [P, T * K], f32) as t,
        nc.sbuf_tensor("junk_a", [P, K], f32) as junk_a,
        nc.sbuf_tensor("sq_v", [P, K], f32) as sq_v,
        nc.sbuf_tensor("s_a", [P, NA], f32) as s_a,
        nc.sbuf_tensor("s_v", [P, NV], f32) as s_v,
        nc.sbuf_tensor("partial", [P, 2], f32) as partial,
        nc.sbuf_tensor("red", [P, 2], f32) as red,
        nc.sbuf_tensor("scratch", [1, 1], f32) as scratch,
    ):
        with nc.Block() as block:

            @block.sync
            def _(sync):
                for i in range(T):
                    sync.dma_start(
                        out=t.ap()[:, i * K : (i + 1) * K],
                        in_=d[i * P : (i + 1) * P, :],
                    ).then_inc(dsem[i], 16)
                if reduce_mode == "gpsimd":
                    sync.wait_ge(pool_sem, 1)
                    out_src = red.ap()[0:1, :]
                else:
                    sync.wait_ge(act_sem, NA + 2)
                    out_src = partial.ap()
                sync.dma_start(out=out, in_=out_src).then_inc(outd_sem, 16)
                if wait_out == "flush":
                    sync.dma_start(out=scratch.ap(), in_=d[0:1, 0:1]).then_inc(
                        dummy_sem, 16
                    )
                    sync.wait_ge(outd_sem, 16)
                elif wait_out == "wait":
                    sync.wait_ge(outd_sem, 16)
                # "none": SP stream just ends; program epilogue drains DMA

            @block.scalar
            def _(scalar):
                for j, i in enumerate(act_tiles):
                    scalar.wait_ge(dsem[i], 16)
                    if j > 0:
                        scalar.wait_ge(act_sem, j)
                    scalar.activation(
                        junk_a.ap(),
                        t.ap()[:, i * K : (i + 1) * K],
                        Square,
                        accum_out=s_a.ap()[:, j : j + 1],
                    ).then_inc(act_sem)
                scalar.wait_ge(act_sem, NA)
                scalar.wait_ge(dve_sem, 2 * NV)
                # sum_j (1 - s_a[p,j])^2 over ACT's tiles
                scalar.activation(
                    junk_a.ap()[:, 0:NA],
                    s_a.ap(),
                    Square,
                    bias=1.0,
                    scale=-1.0,
                    accum_out=partial.ap()[:, 0:1],
                ).then_inc(act_sem)
                scalar.wait_ge(act_sem, NA + 1)
                # s_v holds means: sum_j (1 - K*mean)^2 over DVE's tiles
                scalar.activation(
                    junk_a.ap()[:, NA : NA + NV],
                    s_v.ap(),
                    Square,
                    bias=1.0,
                    scale=-float(K),
                    accum_out=partial.ap()[:, 1:2],
                ).then_inc(act_sem)

            @block.vector
            def _(vector):
                for j, i in enumerate(dve_tiles):
                    vector.wait_ge(dsem[i], 16)
                    if j > 0:
                        vector.wait_ge(dve_sem, 2 * j)
                    vector.tensor_mul(
                        sq_v.ap(),
                        t.ap()[:, i * K : (i + 1) * K],
                        t.ap()[:, i * K : (i + 1) * K],
                    ).then_inc(dve_sem)
                    vector.wait_ge(dve_sem, 2 * j + 1)
                    vector.pool_avg(s_v.ap()[:, j : j + 1], sq_v.ap()).then_inc(
                        dve_sem
                    )

            if reduce_mode == "gpsimd":
                import concourse.bass_isa as bass_isa

                @block.gpsimd
                def _(gpsimd):
                    gpsimd.wait_ge(act_sem, NA + 2)
                    gpsimd.partition_all_reduce(
                        red.ap(),
                        partial.ap(),
                        channels=P,
                        reduce_op=bass_isa.ReduceOp.add,
                    ).then_inc(pool_sem)

    nc.compile()
    return nc


def _build_nc_v4(dve_tiles=(0, 3, 6, 9, 12), wait_out="none", reduce_mode="gpsimd"):
    """Weighted ACT/DVE split (ACT ~2.3us/tile vs DVE ~4.6us/tile), single
    final activation (DVE rescales its pool-averages to sums in place),
    gpsimd cross-partition reduce -> 4-byte output DMA, no completion wait
    (the program epilogue's drain guarantees the write lands before exec
    completes)."""
    import concourse.bass_isa as bass_isa

    f32 = mybir.dt.float32
    nc = bacc.Bacc("TRN2", target_bir_lowering=False, debug=False)
    d = nc.dram_tensor("d", [R, K], f32, kind="ExternalInput").ap()
    out_shape = [1, 1] if reduce_mode == "gpsimd" else [P, 1]
    out = nc.dram_tensor("out", out_shape, f32, kind="ExternalOutput").ap()
    Square = mybir.ActivationFunctionType.Square

    dve_tiles = list(dve_tiles)
    act_tiles = [i for i in range(T) if i not in dve_tiles]
    NA, NV = len(act_tiles), len(dve_tiles)

    ctx = ExitStack()
    dsem = [ctx.enter_context(nc.semaphore(f"dma_{i}")) for i in range(T)]
    with (
        ctx,
        nc.semaphore("act_sem") as act_sem,
        nc.semaphore("dve_sem") as dve_sem,
        nc.semaphore("pool_sem") as pool_sem,
        nc.semaphore("outd_sem") as outd_sem,
        nc.sbuf_tensor("t", [P, T * K], f32) as t,
        nc.sbuf_tensor("junk_a", [P, K], f32) as junk_a,
        nc.sbuf_tensor("sq_v", [P, K], f32) as sq_v,
        nc.sbuf_tensor("s", [P, T], f32) as s,
        nc.sbuf_tensor("partial", [P, 1], f32) as partial,
        nc.sbuf_tensor("red", [P, 1], f32) as red,
    ):
        with nc.Block() as block:

            @block.sync
            def _(sync):
                for i in range(T):
                    sync.dma_start(
                        out=t.ap()[:, i * K : (i + 1) * K],
                        in_=d[i * P : (i + 1) * P, :],
                    ).then_inc(dsem[i], 16)
                if reduce_mode == "gpsimd":
                    sync.wait_ge(pool_sem, 1)
                    out_src = red.ap()[0:1, :]
                else:
                    sync.wait_ge(act_sem, NA + 1)
                    out_src = partial.ap()
                sync.dma_start(out=out, in_=out_src).then_inc(outd_sem, 16)
                if wait_out == "wait":
                    sync.wait_ge(outd_sem, 16)

            @block.scalar
            def _(scalar):
                for j, i in enumerate(act_tiles):
                    scalar.wait_ge(dsem[i], 16)
                    if j > 0:
                        scalar.wait_ge(act_sem, j)
                    scalar.activation(
                        junk_a.ap(),
                        t.ap()[:, i * K : (i + 1) * K],
                        Square,
                        accum_out=s.ap()[:, i : i + 1],
                    ).then_inc(act_sem)
                scalar.wait_ge(act_sem, NA)
                scalar.wait_ge(dve_sem, 3 * NV)
                scalar.activation(
                    junk_a.ap()[:, 0:T],
                    s.ap(),
                    Square,
                    bias=1.0,
                    scale=-1.0,
                    accum_out=partial.ap(),
                ).then_inc(act_sem)

            @block.vector
            def _(vector):
                for j, i in enumerate(dve_tiles):
                    vector.wait_ge(dsem[i], 16)
                    if j > 0:
                        vector.wait_ge(dve_sem, 3 * j)
                    vector.tensor_mul(
                        sq_v.ap(),
                        t.ap()[:, i * K : (i + 1) * K],
                        t.ap()[:, i * K : (i + 1) * K],
                    ).then_inc(dve_sem)
                    vector.wait_ge(dve_sem, 3 * j + 1)
                    vector.pool_avg(s.ap()[:, i : i + 1], sq_v.ap()).then_inc(dve_sem)
                    vector.wait_ge(dve_sem, 3 * j + 2)
                    vector.tensor_scalar_mul(
                        s.ap()[:, i : i + 1], s.ap()[:, i : i + 1], float(K)
                    ).then_inc(dve_sem)

            if reduce_mode == "gpsimd":

                @block.gpsimd
                def _(gpsimd):
                    gpsimd.wait_ge(act_sem, NA + 1)
                    gpsimd.partition_all_reduce(
                        red.ap(),
                        partial.ap(),
                        channels=P,
                        reduce_op=bass_isa.ReduceOp.add,
                    ).then_inc(pool_sem)

    nc.compile()
    return nc


def _build_nc_v5(dve_tiles=(0, 3, 6, 9, 12), reduce_mode="gpsimd"):
    """v4 + the last row-block streams as two half-K DMAs so ACT can start on
    the first half while the second is still in flight; DVE merges the two
    half-sums. Cuts ~1us off the post-DMA tail."""
    import concourse.bass_isa as bass_isa

    f32 = mybir.dt.float32
    nc = bacc.Bacc("TRN2", target_bir_lowering=False, debug=False)
    d = nc.dram_tensor("d", [R, K], f32, kind="ExternalInput").ap()
    out_shape = [1, 1] if reduce_mode == "gpsimd" else [P, 1]
    out = nc.dram_tensor("out", out_shape, f32, kind="ExternalOutput").ap()
    Square = mybir.ActivationFunctionType.Square
    KH = K // 2
    LAST = T - 1

    dve_tiles = list(dve_tiles)
    assert LAST not in dve_tiles
    act_full = [i for i in range(T - 1) if i not in dve_tiles]
    NV = len(dve_tiles)
    NACT = len(act_full) + 2  # + two half-tile activations

    ctx = ExitStack()
    # dma index: i in [0,T-1) -> tile i; T-1 -> last half A; T -> last half B
    dsem = [ctx.enter_context(nc.semaphore(f"dma_{i}")) for i in range(T + 1)]
    with (
        ctx,
        nc.semaphore("act_sem") as act_sem,
        nc.semaphore("dve_sem") as dve_sem,
        nc.semaphore("pool_sem") as pool_sem,
        nc.semaphore("outd_sem") as outd_sem,
        nc.sbuf_tensor("t", [P, T * K], f32) as t,
        nc.sbuf_tensor("junk_a", [P, K], f32) as junk_a,
        nc.sbuf_tensor("sq_v", [P, K], f32) as sq_v,
        nc.sbuf_tensor("s", [P, T + 1], f32) as s,
        nc.sbuf_tensor("partial", [P, 1], f32) as partial,
        nc.sbuf_tensor("red", [P, 1], f32) as red,
    ):
        with nc.Block() as block:

            @block.sync
            def _(sync):
                for i in range(T - 1):
                    sync.dma_start(
                        out=t.ap()[:, i * K : (i + 1) * K],
                        in_=d[i * P : (i + 1) * P, :],
                    ).then_inc(dsem[i], 16)
                for h in range(2):
                    sync.dma_start(
                        out=t.ap()[:, LAST * K + h * KH : LAST * K + (h + 1) * KH],
                        in_=d[LAST * P : (LAST + 1) * P, h * KH : (h + 1) * KH],
                    ).then_inc(dsem[LAST + h], 16)
                if reduce_mode == "gpsimd":
                    sync.wait_ge(pool_sem, 1)
                    out_src = red.ap()[0:1, :]
                else:
                    sync.wait_ge(act_sem, NACT + 1)
                    out_src = partial.ap()
                sync.dma_start(out=out, in_=out_src).then_inc(outd_sem, 16)

            @block.scalar
            def _(scalar):
                na = 0
                for i in act_full:
                    scalar.wait_ge(dsem[i], 16)
                    if na > 0:
                        scalar.wait_ge(act_sem, na)
                    scalar.activation(
                        junk_a.ap(),
                        t.ap()[:, i * K : (i + 1) * K],
                        Square,
                        accum_out=s.ap()[:, i : i + 1],
                    ).then_inc(act_sem)
                    na += 1
                for h in range(2):
                    scalar.wait_ge(dsem[LAST + h], 16)
                    scalar.wait_ge(act_sem, na)
                    scalar.activation(
                        junk_a.ap()[:, 0:KH],
                        t.ap()[:, LAST * K + h * KH : LAST * K + (h + 1) * KH],
                        Square,
                        accum_out=s.ap()[:, LAST + h : LAST + h + 1],
                    ).then_inc(act_sem)
                    na += 1
                scalar.wait_ge(act_sem, NACT)
                scalar.wait_ge(dve_sem, 3 * NV + 1)  # +1: the half-sum merge
                scalar.activation(
                    junk_a.ap()[:, 0:T],
                    s.ap()[:, 0:T],
                    Square,
                    bias=1.0,
                    scale=-1.0,
                    accum_out=partial.ap(),
                ).then_inc(act_sem)

            @block.vector
            def _(vector):
                for j, i in enumerate(dve_tiles):
                    vector.wait_ge(dsem[i], 16)
                    if j > 0:
                        vector.wait_ge(dve_sem, 3 * j)
                    vector.tensor_mul(
                        sq_v.ap(),
                        t.ap()[:, i * K : (i + 1) * K],
                        t.ap()[:, i * K : (i + 1) * K],
                    ).then_inc(dve_sem)
                    vector.wait_ge(dve_sem, 3 * j + 1)
                    vector.pool_avg(s.ap()[:, i : i + 1], sq_v.ap()).then_inc(dve_sem)
                    vector.wait_ge(dve_sem, 3 * j + 2)
                    vector.tensor_scalar_mul(
                        s.ap()[:, i : i + 1], s.ap()[:, i : i + 1], float(K)
                    ).then_inc(dve_sem)
                # merge the two half-sums of the last row block into col LAST
                vector.wait_ge(act_sem, NACT)
                vector.wait_ge(dve_sem, 3 * NV)
                vector.tensor_add(
                    s.ap()[:, LAST : LAST + 1],
                    s.ap()[:, LAST : LAST + 1],
                    s.ap()[:, LAST + 1 : LAST + 2],
                ).then_inc(dve_sem)

            if reduce_mode == "gpsimd":

                @block.gpsimd
                def _(gpsimd):
                    gpsimd.wait_ge(act_sem, NACT + 1)
                    gpsimd.partition_all_reduce(
                        red.ap(),
                        partial.ap(),
                        channels=P,
                        reduce_op=bass_isa.ReduceOp.add,
                    ).then_inc(pool_sem)

    nc.compile()
    return nc


def _get_nc():
    global _nc_cache
    if _nc_cache is None:
        _nc_cache = _build_nc_v5()
    return _nc_cache


def run_shards(d, **spmd_kwargs):
    """Run the SPMD kernel; returns the BassKernelResults (for profiling)."""
    d = np.ascontiguousarray(np.asarray(d, dtype=np.float32))
    assert d.shape == (N, K), d.shape
    shards = d.reshape(NCORES, R, K)
    in_maps = [{"d": shards[c]} for c in range(NCORES)]
    return run_bass_kernel_spmd(_get_nc(), in_maps, list(range(NCORES)), **spmd_kwargs)


def _combine(results):
    total = 0.0
    for r in results:
        total += float(np.sum(r["out"].astype(np.float64)))
    return np.float32(0.001 * np.sqrt(total))


def kernel(d):
    return _combine(run_shards(d).results)


# revision 30
# speedup vs baseline: 1.0152x; 1.0152x over previous
"""Bass/Trainium2 kernel for nn_D_constraint1: 0.001*sqrt(sum_i (||d_i||^2 - 1)^2).

Sharding: d [16384, 2048] fp32 is split row-wise across 8 NeuronCores (2048
rows per core, as the sharding hint suggests). Each core streams its 16 MiB
shard HBM->SBUF once (the kernel is HBM-bandwidth bound at ~400 GB/s/core)
as 15 full [128,2048] row-block tiles plus two half-K tiles for the final
row block, and computes per-row sums of squares on the fly:

  - scalar engine (ACT): Square activation with the free-axis accumulator
    (one pass per tile, ~2.3us) on 10 full tiles + the last half-tile;
  - vector engine (DVE): square via tensor_mul then pool_avg (+rescale) on
    5 interleaved tiles + the first half of the last row block, so the two
    engines drain the tail in parallel with the DMA stream.

A final ACT activation computes sum_j (1 - s_j)^2 per partition, GpSimd
all-reduces across partitions, and a single 4-byte DMA writes the per-core
scalar out. No completion wait is needed: the program epilogue's drain
guarantees the write lands before execution completes.

The host gathers the 8 per-core scalars, sums, takes sqrt and scales --
the scalar "all-reduce" of the sharding hint.

First two tile DMAs issue from the scalar engine's HW-DGE ring (it exits the
init barrier slightly before SP reaches its first descriptor); the rest from
the sync engine's ring. Each DMA gets its own semaphore: completion counts of
different in-flight DMAs interleave, so shared-semaphore thresholds cannot
tell *which* transfer finished.
"""

from contextlib import ExitStack

import numpy as np

import concourse.bass as bass
from concourse import bacc, mybir
from concourse.bass_utils import run_bass_kernel_spmd

N, K = 16384, 2048
NCORES = 8
R = N // NCORES  # rows per core
P = 128          # SBUF partitions
T = R // P       # row-tiles per core

_nc_cache = None


def _build_nc(dve_tiles=(0, 3, 6, 9, 12)):
    import concourse.bass_isa as bass_isa

    f32 = mybir.dt.float32
    nc = bacc.Bacc("TRN2", target_bir_lowering=False, debug=False)
    d = nc.dram_tensor("d", [R, K], f32, kind="ExternalInput").ap()
    out = nc.dram_tensor("out", [1, 1], f32, kind="ExternalOutput").ap()
    Square = mybir.ActivationFunctionType.Square
    KH = K // 2
    LAST = T - 1

    dve_tiles = list(dve_tiles)
    assert LAST not in dve_tiles and LAST - 1 not in dve_tiles
    act_full = [i for i in range(T - 1) if i not in dve_tiles]
    NV = len(dve_tiles)
    NACT = len(act_full) + 1  # + half2
    NDVE = 3 * NV + 3         # + half1 mul/pool/rescale; merge is NDVE+1

    ctx = ExitStack()
    # dma index: tiles 0..T-2 full; T-1 -> half1; T -> half2 of last block
    dsem = [ctx.enter_context(nc.semaphore(f"dma_{i}")) for i in range(T + 1)]
    with (
        ctx,
        nc.semaphore("act_sem") as act_sem,
        nc.semaphore("dve_sem") as dve_sem,
        nc.semaphore("pool_sem") as pool_sem,
        nc.semaphore("outd_sem") as outd_sem,
        nc.sbuf_tensor("t", [P, T * K], f32) as t,
        nc.sbuf_tensor("junk_a", [P, K], f32) as junk_a,
        nc.sbuf_tensor("sq_v", [P, K], f32) as sq_v,
        # cols 0..T-1: per-tile row sums (col LAST merged); col T: half2 sum
        nc.sbuf_tensor("s", [P, T + 1], f32) as s,
        nc.sbuf_tensor("partial", [P, 1], f32) as partial,
        nc.sbuf_tensor("red", [P, 1], f32) as red,
    ):
        with nc.Block() as block:

            @block.sync
            def _(sync):
                for i in range(2, T - 1):
                    sync.dma_start(
                        out=t.ap()[:, i * K : (i + 1) * K],
                        in_=d[i * P : (i + 1) * P, :],
                    ).then_inc(dsem[i], 16)
                for h in range(2):
                    sync.dma_start(
                        out=t.ap()[:, LAST * K + h * KH : LAST * K + (h + 1) * KH],
                        in_=d[LAST * P : (LAST + 1) * P, h * KH : (h + 1) * KH],
                    ).then_inc(dsem[LAST + h], 16)
                sync.wait_ge(pool_sem, 1)
                sync.dma_start(out=out, in_=red.ap()[0:1, :]).then_inc(outd_sem, 16)

            @block.scalar
            def _(scalar):
                for i in range(2):
                    scalar.dma_start(
                        out=t.ap()[:, i * K : (i + 1) * K],
                        in_=d[i * P : (i + 1) * P, :],
                    ).then_inc(dsem[i], 16)
                na = 0
                for i in act_full:
                    scalar.wait_ge(dsem[i], 16)
                    if na > 0:
                        scalar.wait_ge(act_sem, na)
                    scalar.activation(
                        junk_a.ap(),
                        t.ap()[:, i * K : (i + 1) * K],
                        Square,
                        accum_out=s.ap()[:, i : i + 1],
                    ).then_inc(act_sem)
                    na += 1
                # second k-half of the last row block
                scalar.wait_ge(dsem[LAST + 1], 16)
                scalar.wait_ge(act_sem, na)
                scalar.activation(
                    junk_a.ap()[:, 0:KH],
                    t.ap()[:, LAST * K + KH : (LAST + 1) * K],
                    Square,
                    accum_out=s.ap()[:, T : T + 1],
                ).then_inc(act_sem)
                scalar.wait_ge(act_sem, NACT)
                scalar.wait_ge(dve_sem, NDVE + 1)
                # partial[p] = sum_j (1 - s[p,j])^2
                scalar.activation(
                    junk_a.ap()[:, 0:T],
                    s.ap()[:, 0:T],
                    Square,
                    bias=1.0,
                    scale=-1.0,
                    accum_out=partial.ap(),
                ).then_inc(act_sem)

            @block.vector
            def _(vector):
                nv = 0
                for i in dve_tiles:
                    vector.wait_ge(dsem[i], 16)
                    if nv > 0:
                        vector.wait_ge(dve_sem, nv)
                    vector.tensor_mul(
                        sq_v.ap(),
                        t.ap()[:, i * K : (i + 1) * K],
                        t.ap()[:, i * K : (i + 1) * K],
                    ).then_inc(dve_sem)
                    vector.wait_ge(dve_sem, nv + 1)
                    vector.pool_avg(s.ap()[:, i : i + 1], sq_v.ap()).then_inc(dve_sem)
                    vector.wait_ge(dve_sem, nv + 2)
                    vector.tensor_scalar_mul(
                        s.ap()[:, i : i + 1], s.ap()[:, i : i + 1], float(K)
                    ).then_inc(dve_sem)
                    nv += 3
                # first k-half of the last row block
                vector.wait_ge(dsem[LAST], 16)
                vector.wait_ge(dve_sem, nv)
                vector.tensor_mul(
                    sq_v.ap()[:, 0:KH],
                    t.ap()[:, LAST * K : LAST * K + KH],
                    t.ap()[:, LAST * K : LAST * K + KH],
                ).then_inc(dve_sem)
                vector.wait_ge(dve_sem, nv + 1)
                vector.pool_avg(
                    s.ap()[:, LAST : LAST + 1], sq_v.ap()[:, 0:KH]
                ).then_inc(dve_sem)
                vector.wait_ge(dve_sem, nv + 2)
                vector.tensor_scalar_mul(
                    s.ap()[:, LAST : LAST + 1], s.ap()[:, LAST : LAST + 1], float(KH)
                ).then_inc(dve_sem)
                # merge: s[LAST] += s[T] (ACT's half2 sum)
                vector.wait_ge(act_sem, NACT)
                vector.wait_ge(dve_sem, NDVE)
                vector.tensor_add(
                    s.ap()[:, LAST : LAST + 1],
                    s.ap()[:, LAST : LAST + 1],
                    s.ap()[:, T : T + 1],
                ).then_inc(dve_sem)

            @block.gpsimd
            def _(gpsimd):
                gpsimd.wait_ge(act_sem, NACT + 1)
                gpsimd.partition_all_reduce(
                    red.ap(),
                    partial.ap(),
                    channels=P,
                    reduce_op=bass_isa.ReduceOp.add,
                ).then_inc(pool_sem)

    nc.compile()
    return nc


def _get_nc():
    global _nc_cache
    if _nc_cache is None:
        _nc_cache = _build_nc()
    return _nc_cache


def run_shards(d, **spmd_kwargs):
    """Run the SPMD kernel; returns the BassKernelResults (for profiling)."""
    d = np.ascontiguousarray(np.asarray(d, dtype=np.float32))
    assert d.shape == (N, K), d.shape
    shards = d.reshape(NCORES, R, K)
    in_maps = [{"d": shards[c]} for c in range(NCORES)]
    nc = _get_nc()
    try:
        return run_bass_kernel_spmd(nc, in_maps, list(range(NCORES)), **spmd_kwargs)
    except Exception:
        # A crashed predecessor process can leave a core transiently
        # "unrecoverable"; one retry after re-touching the devices clears it.
        import jax

        jax.devices()
        return run_bass_kernel_spmd(nc, in_maps, list(range(NCORES)), **spmd_kwargs)


def _combine(results):
    total = 0.0
    for r in results:
        total += float(np.sum(r["out"].astype(np.float64)))
    return np.float32(0.001 * np.sqrt(total))


def kernel(d):
    return _combine(run_shards(d).results)
